# revision 5
# baseline (speedup 1.0000x reference)
"""Trainium2 Bass kernel for the DeepBSDE loss (nn_BaseDeepBSDE).

Data-parallel over 8 NeuronCores: each core simulates 2048 Monte-Carlo
paths through the 100-step SDE loop and produces a partial loss sum;
the host gathers the 8 partial scalars.

Device-side layout (per core, Bc = 2048 paths):
  - "folded" state layout: [128 partitions, 16] with path b = c*128 + p
  - MLP activations feature-major: [128 features (z-MLP 0:63 | q-MLP
    64:127), batch free-dim], bf16 matmuls with fp32 PSUM accumulate
  - y state kept as [16, 128] row-chunks so layer-1 runs as K=128
    block matmuls straight from SBUF
  - noise tensors pre-folded on host to [128, steps*48] so every
    per-step slice is a contiguous SBUF view (no per-step DMA)
  - no fp32 matmuls inside the step loop (they would disable FWL);
    loss accumulates via ACT accum_out into an SBUF [128,1] column
"""

import os
import sys

sys.path.insert(0, "/opt/trn_rl_repo")

import numpy as np

B = 16384
NSTEPS = 100
DIMW = 3
DT = 0.01
SQRT_DT = DT**0.5
SIGMA0 = 0.5
NCORES = 8
BC = B // NCORES  # 2048 paths per core
NCH = BC // 128  # 16 chunks of 128 paths
NQ = 4  # noise quarter-buffers

LAST_EXEC_NS = None
LAST_RESULTS = None

_CACHE = {}


def _build(nsteps, debug=False):
    import concourse.tile as tile
    from concourse import bacc, mybir

    f32 = mybir.dt.float32
    bf16 = mybir.dt.bfloat16
    AF = mybir.ActivationFunctionType
    ALU = mybir.AluOpType
    AX = mybir.AxisListType

    nc = bacc.Bacc("TRN2", target_bir_lowering=False, debug=False, num_devices=NCORES)

    # ---------------- DRAM I/O ----------------
    QSTEPS = (nsteps + NQ - 1) // NQ  # steps per noise quarter-buffer
    dWf_d = [
        nc.dram_tensor(f"dWf{q}", [128, QSTEPS * 48], f32, kind="ExternalInput").ap()
        for q in range(NQ)
    ]
    dZf_d = [
        nc.dram_tensor(f"dZf{q}", [128, QSTEPS * 48], f32, kind="ExternalInput").ap()
        for q in range(NQ)
    ]
    L1b_d = nc.dram_tensor("L1b", [128, NCH * 128], f32, kind="ExternalInput").ap()
    W1cT_d = nc.dram_tensor("W1cT", [128, 2], f32, kind="ExternalInput").ap()
    W2bd_d = nc.dram_tensor("W2bd", [128, 128], f32, kind="ExternalInput").ap()
    W3c_d = nc.dram_tensor("W3c", [128, 4], f32, kind="ExternalInput").ap()
    b1c_d = nc.dram_tensor("b1c", [128, 1], f32, kind="ExternalInput").ap()
    b2c_d = nc.dram_tensor("b2c", [128, 1], f32, kind="ExternalInput").ap()
    b3c_d = nc.dram_tensor("b3c", [1, 4], f32, kind="ExternalInput").ap()
    trep_d = nc.dram_tensor("trep", [128, nsteps], f32, kind="ExternalInput").ap()
    ones_col_d = nc.dram_tensor("ones_col", [128, 1], f32, kind="ExternalInput").ap()
    ones_row_d = nc.dram_tensor("ones_row", [1, 128], f32, kind="ExternalInput").ap()
    I128_d = nc.dram_tensor("I128", [128, 128], f32, kind="ExternalInput").ap()
    y_init_d = nc.dram_tensor("y_init", [16, 128], f32, kind="ExternalInput").ap()
    Y_init_d = nc.dram_tensor("Y_init", [128, 16], f32, kind="ExternalInput").ap()

    loss_out = nc.dram_tensor("loss_out", [1, 1], f32, kind="ExternalOutput").ap()
    if debug:
        y_out = nc.dram_tensor("y_out", [16, 128], f32, kind="ExternalOutput").ap()
        Y_out = nc.dram_tensor("Y_out", [128, 16], f32, kind="ExternalOutput").ap()
        zq_out = nc.dram_tensor("zq_out", [128, 64], f32, kind="ExternalOutput").ap()

    with tile.TileContext(nc) as tc:
        from contextlib import ExitStack

        with ExitStack() as ctx:
            cpool = ctx.enter_context(tc.tile_pool(name="const", bufs=1))
            h1pool = ctx.enter_context(tc.tile_pool(name="h1sb", bufs=2))
            h2pool = ctx.enter_context(tc.tile_pool(name="h2sb", bufs=2))
            pmm = ctx.enter_context(tc.tile_pool(name="pmm", bufs=3, space="PSUM"))
            pzq = ctx.enter_context(tc.tile_pool(name="pzq", bufs=1, space="PSUM"))
            ptr = ctx.enter_context(tc.tile_pool(name="ptr", bufs=1, space="PSUM"))

            # ------------- persistent SBUF tiles -------------
            dWs = [cpool.tile([128, QSTEPS * 48], f32, tag=f"dw{q}", name=f"dws{q}") for q in range(NQ)]
            dZs = [cpool.tile([128, QSTEPS * 48], f32, tag=f"dz{q}", name=f"dzs{q}") for q in range(NQ)]
            swp = cpool.tile([128, nsteps * 16], f32, tag="swp")
            L1b_bf = cpool.tile([128, NCH * 128], bf16, tag="l1b")
            W2bd_bf = cpool.tile([128, 128], bf16, tag="w2bd")
            W3_bf = cpool.tile([128, 4], bf16, tag="w3")
            W3_f = cpool.tile([128, 4], f32, tag="w3f")
            b1tab = cpool.tile([128, nsteps], f32, tag="b1tab")
            W1cT_sb = cpool.tile([128, 2], f32, tag="w1ct")
            trep = cpool.tile([128, nsteps], f32, tag="trep")
            b1c_sb = cpool.tile([128, 1], f32, tag="b1c")
            b2c_sb = cpool.tile([128, 1], f32, tag="b2c")
            b3s = cpool.tile([1, 4], f32, tag="b3s")
            b3f = cpool.tile([1, 4], f32, tag="b3f")
            b3rep = cpool.tile([1, 64], bf16, tag="b3rep")
            b3tile = cpool.tile([128, 64], f32, tag="b3tile")
            ones_bf = cpool.tile([1, 128], bf16, tag="ones_bf")
            ones_col = cpool.tile([128, 1], f32, tag="ones_col")
            I128 = cpool.tile([128, 128], f32, tag="i128")
            I128_bf = cpool.tile([128, 128], bf16, tag="i128bf")
            y16 = cpool.tile([16, 128], f32, tag="y16")
            y16pad = cpool.tile([128, 128], bf16, tag="y16pad")
            Y_f = cpool.tile([128, 16], f32, tag="Yf")
            zqf_sb = cpool.tile([128, 64], f32, tag="zqf_sb")
            zz = cpool.tile([128, 96], f32, tag="zz")
            uv = cpool.tile([128, 32], f32, tag="uv")
            r_t = cpool.tile([128, 16], f32, tag="r")
            rr_t = cpool.tile([128, 16], f32, tag="rr")
            rsum = cpool.tile([128, 1], f32, tag="rsum")
            loss_acc = cpool.tile([128, 1], f32, tag="loss_acc")
            incr = cpool.tile([128, 16], bf16, tag="incr")
            fDT = cpool.tile([128, 16], f32, tag="fdt")
            umf = cpool.tile([128, 16], f32, tag="umf")
            ysq16 = cpool.tile([16, 128], f32, tag="ysq16")
            ee = cpool.tile([128, 16], f32, tag="ee")
            esum = cpool.tile([128, 1], f32, tag="esum")
            loss1 = cpool.tile([1, 1], f32, tag="loss1")

            # ------------- init: DMAs -------------
            for q in range(NQ):
                nc.sync.dma_start(dWs[q][:], dWf_d[q][:])
                nc.sync.dma_start(dZs[q][:], dZf_d[q][:])
            # f32 -> bf16 cast during DMA (SWDGE)
            nc.gpsimd.dma_start(L1b_bf[:], L1b_d[:])
            nc.gpsimd.dma_start(W2bd_bf[:], W2bd_d[:])
            nc.gpsimd.dma_start(ones_bf[:], ones_row_d[:])
            nc.gpsimd.dma_start(I128_bf[:], I128_d[:])
            nc.sync.dma_start(W3_f[:], W3c_d[:])
            nc.sync.dma_start(W1cT_sb[:], W1cT_d[:])
            nc.sync.dma_start(trep[:], trep_d[:])
            nc.sync.dma_start(b1c_sb[:], b1c_d[:])
            nc.sync.dma_start(b2c_sb[:], b2c_d[:])
            nc.sync.dma_start(b3f[:], b3c_d[:])
            nc.sync.dma_start(ones_col[:], ones_col_d[:])
            nc.sync.dma_start(I128[:], I128_d[:])
            nc.sync.dma_start(y16[:], y_init_d[:])
            nc.sync.dma_start(Y_f[:], Y_init_d[:])

            # ------------- init: compute (no fp32 matmuls) -------------
            # b1tab[:, i] = t_i * W1row0[:] + b1c   (DVE, exact fp32)
            nc.vector.tensor_scalar(
                b1tab[:], trep[:], W1cT_sb[:, 0:1], b1c_sb[:, 0:1],
                op0=ALU.mult, op1=ALU.add,
            )
            # W3 scaling: z-cols * sqrt(dt), q-col * dt  (cast to bf16)
            nc.vector.tensor_scalar_mul(W3_bf[:, 0:3], W3_f[:, 0:3], float(SQRT_DT))
            nc.vector.tensor_scalar_mul(W3_bf[:, 3:4], W3_f[:, 3:4], float(DT))
            # b3 scaling + replicate x16 (bf16 row for matmul + f32 tile)
            nc.vector.tensor_scalar_mul(b3s[0:1, 0:3], b3f[0:1, 0:3], float(SQRT_DT))
            nc.vector.tensor_scalar_mul(b3s[0:1, 3:4], b3f[0:1, 3:4], float(DT))
            nc.vector.tensor_copy(b3rep[0:1, 0:4], b3s[0:1, :])
            nc.vector.tensor_copy(b3rep[0:1, 4:8], b3rep[0:1, 0:4])
            nc.vector.tensor_copy(b3rep[0:1, 8:16], b3rep[0:1, 0:8])
            nc.vector.tensor_copy(b3rep[0:1, 16:32], b3rep[0:1, 0:16])
            nc.vector.tensor_copy(b3rep[0:1, 32:64], b3rep[0:1, 0:32])
            # b3tile[p, :] = b3rep (partition broadcast via one bf16 matmul)
            b3ps = pzq.tile([128, 64], f32, tag="zq", name="b3ps")
            nc.tensor.matmul(b3ps[:], ones_bf[0:1, :], b3rep[0:1, :], start=True, stop=True)
            nc.scalar.copy(b3tile[:], b3ps[:])

            nc.vector.memset(y16pad[:], 0.0)
            nc.vector.memset(loss_acc[:], 0.0)

            # sw prepass: swp[:, i*16+c] = sigma0*sqrt(dt) * sum_j dW[i,c*128+p,j]
            for q in range(NQ):
                nsq = max(0, min(nsteps, (q + 1) * QSTEPS) - q * QSTEPS)
                if nsq == 0:
                    continue
                lo = q * QSTEPS * 16
                src = dWs[q][:, 0 : nsq * 48].rearrange("p (s j) -> p s j", j=3)
                nc.vector.tensor_reduce(
                    swp[:, lo : lo + nsq * 16], src, axis=AX.X, op=ALU.add
                )
            nc.vector.tensor_scalar_mul(swp[:], swp[:], float(SIGMA0 * SQRT_DT))

            # ------------- time-step loop -------------
            SC_F = float((0.5 / DT) ** 0.5)  # fDT = (SC_F * qDT)^2 = 0.5*dt*q^2
            for i in range(nsteps):
                qi, ri = divmod(i, QSTEPS)
                dwf_i = dWs[qi][:, ri * 48 : (ri + 1) * 48].rearrange(
                    "p (c j) -> p c j", j=3
                )
                dzf_i = dZs[qi][:, ri * 48 : (ri + 1) * 48].rearrange(
                    "p (c j) -> p c j", j=3
                )

                # y -> bf16 padded rhs (GpSimd, SBUF only)
                nc.gpsimd.tensor_copy(y16pad[0:16, :], y16[:])

                # L1: h1[f, b] = W1row1[f] * y[b]  (bias added in relu copy)
                h1ps = [pmm.tile([128, 1024], f32, tag="mm", name=f"h1ps{i}_{k}") for k in range(2)]
                for c in range(NCH):
                    s, o = divmod(c, 8)
                    nc.tensor.matmul(
                        h1ps[s][:, o * 128 : (o + 1) * 128],
                        L1b_bf[:, c * 128 : (c + 1) * 128],
                        y16pad[:],
                        start=True,
                        stop=True,
                    )

                # relu1 (+ per-step bias) -> bf16; one half ACT, one half DVE
                h1sb = h1pool.tile([128, 2048], bf16, tag="h1")
                nc.scalar.activation(
                    h1sb[:, 0:1024], h1ps[0][:], AF.Relu, bias=b1tab[:, i : i + 1]
                )
                nc.vector.tensor_scalar(
                    h1sb[:, 1024:2048], h1ps[1][:], b1tab[:, i : i + 1], 0.0,
                    op0=ALU.add, op1=ALU.max,
                )

                # L2 (2 matmuls, N=1024)
                h2ps = [pmm.tile([128, 1024], f32, tag="mm", name=f"h2ps{i}_{k}") for k in range(2)]
                for s in range(4):
                    nc.tensor.matmul(
                        h2ps[s // 2][:, (s % 2) * 512 : (s % 2 + 1) * 512],
                        W2bd_bf[:],
                        h1sb[:, s * 512 : (s + 1) * 512],
                        start=True,
                        stop=True,
                    )

                # relu2 -> bf16
                h2sb = h2pool.tile([128, 2048], bf16, tag="h2")
                nc.scalar.activation(
                    h2sb[:, 0:1024], h2ps[0][:], AF.Relu, bias=b2c_sb[:, 0:1]
                )
                nc.vector.tensor_scalar(
                    h2sb[:, 1024:2048], h2ps[1][:], b2c_sb[:, 0:1], 0.0,
                    op0=ALU.add, op1=ALU.max,
                )

                # L3 transposed: zqf[p, c*4+m] = sum_f h2[f, c*128+p] * W3s[f, m]
                zqf_ps = pzq.tile([128, 64], f32, tag="zq", name=f"zqps{i}")
                for c in range(NCH):
                    nc.tensor.matmul(
                        zqf_ps[:, c * 4 : (c + 1) * 4],
                        h2sb[:, c * 128 : (c + 1) * 128],
                        W3_bf[:],
                        start=True,
                        stop=True,
                    )
                # psum->sbuf with column bias add
                nc.vector.tensor_tensor(zqf_sb[:], zqf_ps[:], b3tile[:], op=ALU.add)

                # epilogue (folded [128, 16*k] tiles)
                zview = zqf_sb[:].rearrange("p (c m) -> p c m", m=4)[:, :, 0:3]
                qview = zqf_sb[:].rearrange("p (c m) -> p c m", m=4)[:, :, 3:4]
                zz0 = zz[:, 0:48].rearrange("p (c j) -> p c j", j=3)
                zz1 = zz[:, 48:96].rearrange("p (c j) -> p c j", j=3)
                nc.gpsimd.tensor_tensor(zz0, zview, dwf_i, op=ALU.mult)
                nc.gpsimd.tensor_tensor(zz1, zview, dzf_i, op=ALU.mult)
                nc.vector.tensor_reduce(
                    uv[:],
                    zz[:].rearrange("p (h j) -> p h j", j=3),
                    axis=AX.X,
                    op=ALU.add,
                )
                # r = u - v ; loss_acc += sum_c r^2
                nc.vector.tensor_tensor(
                    r_t[:], uv[:, 0:16], uv[:, 16:32], op=ALU.subtract
                )
                nc.scalar.activation(rr_t[:], r_t[:], AF.Square, accum_out=rsum[:])
                nc.gpsimd.tensor_tensor(loss_acc[:], loss_acc[:], rsum[:], op=ALU.add)
                # y update: y += dt*q + sigma*sqrt(dt)*sum_j dW   (bf16 transpose)
                nc.vector.tensor_tensor(
                    incr[:],
                    qview,
                    swp[:, i * 16 : (i + 1) * 16].rearrange("p (c o) -> p c o", o=1),
                    op=ALU.add,
                )
                incr16 = ptr.tile([16, 128], bf16, tag="tr", name=f"tr{i}")
                nc.tensor.matmul(incr16[:], incr[:], I128_bf[:], is_transpose=True)
                nc.vector.tensor_tensor(y16[:], y16[:], incr16[:], op=ALU.add)
                # Y update: Y += u - 0.5*dt*q^2
                nc.scalar.activation(fDT[:], qview, AF.Square, scale=SC_F)
                nc.vector.tensor_tensor(umf[:], uv[:, 0:16], fDT[:], op=ALU.subtract)
                nc.vector.tensor_tensor(Y_f[:], Y_f[:], umf[:], op=ALU.add)

            # ------------- terminal loss (fp32 matmuls OK here) -------------
            nc.scalar.activation(ysq16[:], y16[:], AF.Square)
            ysq_ps = pzq.tile([128, 16], f32, tag="zq", name="ysqps")
            nc.tensor.matmul(ysq_ps[:], ysq16[:], I128[0:16, 0:16], is_transpose=True)
            nc.vector.tensor_tensor(ee[:], Y_f[:], ysq_ps[:], op=ALU.subtract)
            nc.scalar.activation(ee[:], ee[:], AF.Square, accum_out=esum[:])
            nc.vector.tensor_tensor(loss_acc[:], loss_acc[:], esum[:], op=ALU.add)
            lps = ptr.tile([1, 1], f32, tag="tr", name="lps")
            nc.tensor.matmul(lps[:], ones_col[:], loss_acc[:], start=True, stop=True)
            nc.vector.tensor_scalar_mul(loss1[:], lps[:], 1.0 / B)
            nc.sync.dma_start(loss_out[:], loss1[:])
            if debug:
                nc.sync.dma_start(y_out[:], y16[:])
                nc.sync.dma_start(Y_out[:], Y_f[:])
                nc.sync.dma_start(zq_out[:], zqf_sb[:])

    nc.compile()
    return nc


def _host_inputs(nsteps, y0, Y0, zW1, zb1, zW2, zb2, zW3, zb3, qW1, qb1, qW2, qb2, qW3, qb3, dW, dZ):
    """Per-core input maps. Layout/slicing only — no arithmetic on inputs."""
    f = np.float32
    QSTEPS = (nsteps + NQ - 1) // NQ
    W1row1 = np.concatenate([zW1[1], qW1[1]]).astype(f)  # (128,)
    L1b = np.zeros((128, NCH * 128), f)
    for c in range(NCH):
        L1b[c, c * 128 : (c + 1) * 128] = W1row1
    W1cT = np.ascontiguousarray(np.concatenate([zW1, qW1], axis=1).T).astype(f)  # (128,2)
    W2bd = np.zeros((128, 128), f)
    W2bd[0:64, 0:64] = zW2
    W2bd[64:128, 64:128] = qW2
    W3c = np.zeros((128, 4), f)
    W3c[0:64, 0:3] = zW3
    W3c[64:128, 3] = qW3[:, 0]
    b1c = np.concatenate([zb1, qb1]).astype(f).reshape(128, 1)
    b2c = np.concatenate([zb2, qb2]).astype(f).reshape(128, 1)
    b3c = np.concatenate([zb3, qb3]).astype(f).reshape(1, 4)
    trep = np.broadcast_to((np.arange(nsteps) * DT).astype(f), (128, nsteps)).copy()
    ones_col = np.ones((128, 1), f)
    ones_row = np.ones((1, 128), f)
    I128 = np.eye(128, dtype=f)
    y_init = np.broadcast_to(np.asarray(y0, f).reshape(1, 1), (16, 128)).copy()
    Y_init = np.broadcast_to(np.asarray(Y0, f).reshape(1, 1), (128, 16)).copy()

    shared = dict(
        L1b=L1b, W1cT=W1cT, W2bd=W2bd, W3c=W3c, b1c=b1c, b2c=b2c, b3c=b3c,
        trep=trep, ones_col=ones_col, ones_row=ones_row, I128=I128,
        y_init=y_init, Y_init=Y_init,
    )

    in_maps = []
    for core in range(NCORES):
        o = core * BC
        m = dict(shared)
        for name, arr in (("dWf", dW), ("dZf", dZ)):
            # fold: [nsteps, 2048, 3] -> [128, nsteps*48],
            # col = i*48 + c*3 + j, path = c*128 + p
            x = np.ascontiguousarray(arr[:nsteps, o : o + BC, :]).astype(f)
            x = x.reshape(nsteps, NCH, 128, 3).transpose(2, 0, 1, 3)
            x = np.ascontiguousarray(x).reshape(128, nsteps * 48)
            for q in range(NQ):
                sl = x[:, q * QSTEPS * 48 : (q + 1) * QSTEPS * 48]
                buf = np.zeros((128, QSTEPS * 48), f)
                buf[:, : sl.shape[1]] = sl
                m[f"{name}{q}"] = buf
        in_maps.append(m)
    return in_maps


def _run(nsteps, inputs, debug=False):
    global LAST_EXEC_NS, LAST_RESULTS
    from concourse import bass_utils

    key = (nsteps, debug)
    if key not in _CACHE:
        _CACHE[key] = _build(nsteps, debug=debug)
    nc = _CACHE[key]

    in_maps = _host_inputs(nsteps, **inputs)
    trace = bool(os.environ.get("BASS_TRACE"))
    kwargs = {}
    if trace:
        import tempfile

        kwargs = dict(trace=True, tmpdir=tempfile.mkdtemp(prefix="bsde_trace_"))
    res = bass_utils.run_bass_kernel_spmd(
        nc, in_maps, core_ids=list(range(NCORES)), **kwargs
    )
    LAST_RESULTS = res
    LAST_EXEC_NS = res.exec_time_ns
    return res


def kernel(**inputs):
    inputs = {k: np.asarray(v, np.float32) for k, v in inputs.items()}
    res = _run(NSTEPS, inputs, debug=False)
    total = np.float32(0.0)
    for core in range(NCORES):
        total += res.results[core]["loss_out"][0, 0]
    return np.array(total, dtype=np.float32)


# revision 6
# speedup vs baseline: 1.1082x; 1.1082x over previous
"""Trainium2 Bass kernel for the DeepBSDE loss (nn_BaseDeepBSDE).

Data-parallel over 8 NeuronCores: each core simulates 2048 Monte-Carlo
paths through the 100-step SDE loop and produces a partial loss sum;
the host gathers the 8 partial scalars.

Per core, the 2048 paths are split into TWO independent groups of 1024
(chunks 0-7 and 8-15). Each group carries its own y-state recurrence so
the two per-step serial chains pipeline against each other across all
engines. The Y/loss bookkeeping dangles off the recurrence and runs
merged. Layouts:
  - folded state: [128 partitions, 16] with path b = c*128 + p
  - MLP activations feature-major [128 feat, batch], bf16 matmuls
  - y state as [8, 128] row-chunks per group (K=8 block matmuls)
  - noise pre-folded on host to [128, steps*48]; no per-step DMA
  - no fp32 matmuls inside the step loop
"""

import os
import sys

sys.path.insert(0, "/opt/trn_rl_repo")

import numpy as np

B = 16384
NSTEPS = 100
DIMW = 3
DT = 0.01
SQRT_DT = DT**0.5
SIGMA0 = 0.5
NCORES = 8
BC = B // NCORES  # 2048 paths per core
NCH = BC // 128  # 16 chunks of 128 paths
NG = 2  # independent path groups per core
GCH = NCH // NG  # 8 chunks per group
NQ = 4  # noise quarter-buffers

LAST_EXEC_NS = None
LAST_RESULTS = None

_CACHE = {}


def _build(nsteps, debug=False):
    import concourse.tile as tile
    from concourse import bacc, mybir

    f32 = mybir.dt.float32
    bf16 = mybir.dt.bfloat16
    AF = mybir.ActivationFunctionType
    ALU = mybir.AluOpType
    AX = mybir.AxisListType

    nc = bacc.Bacc("TRN2", target_bir_lowering=False, debug=False, num_devices=NCORES)

    # ---------------- DRAM I/O ----------------
    QSTEPS = (nsteps + NQ - 1) // NQ
    dWf_d = [
        nc.dram_tensor(f"dWf{q}", [128, QSTEPS * 48], f32, kind="ExternalInput").ap()
        for q in range(NQ)
    ]
    dZf_d = [
        nc.dram_tensor(f"dZf{q}", [128, QSTEPS * 48], f32, kind="ExternalInput").ap()
        for q in range(NQ)
    ]
    L1b_d = [
        nc.dram_tensor(f"L1b{g}", [GCH, GCH * 128], f32, kind="ExternalInput").ap()
        for g in range(NG)
    ]
    W1cT_d = nc.dram_tensor("W1cT", [128, 2], f32, kind="ExternalInput").ap()
    W2bd_d = nc.dram_tensor("W2bd", [128, 128], f32, kind="ExternalInput").ap()
    W3c_d = nc.dram_tensor("W3c", [128, 4], f32, kind="ExternalInput").ap()
    b1c_d = nc.dram_tensor("b1c", [128, 1], f32, kind="ExternalInput").ap()
    b2c_d = nc.dram_tensor("b2c", [128, 1], f32, kind="ExternalInput").ap()
    b3c_d = nc.dram_tensor("b3c", [1, 4], f32, kind="ExternalInput").ap()
    trep_d = nc.dram_tensor("trep", [128, nsteps], f32, kind="ExternalInput").ap()
    ones_col_d = nc.dram_tensor("ones_col", [128, 1], f32, kind="ExternalInput").ap()
    ones_row_d = nc.dram_tensor("ones_row", [1, 128], f32, kind="ExternalInput").ap()
    I128_d = nc.dram_tensor("I128", [128, 128], f32, kind="ExternalInput").ap()
    y_init_d = nc.dram_tensor("y_init", [GCH * NG, 128], f32, kind="ExternalInput").ap()
    Y_init_d = nc.dram_tensor("Y_init", [128, 16], f32, kind="ExternalInput").ap()

    loss_out = nc.dram_tensor("loss_out", [1, 1], f32, kind="ExternalOutput").ap()
    if debug:
        y_out = nc.dram_tensor("y_out", [16, 128], f32, kind="ExternalOutput").ap()
        Y_out = nc.dram_tensor("Y_out", [128, 16], f32, kind="ExternalOutput").ap()
        zq_out = nc.dram_tensor("zq_out", [128, 64], f32, kind="ExternalOutput").ap()

    with tile.TileContext(nc) as tc:
        from contextlib import ExitStack

        with ExitStack() as ctx:
            cpool = ctx.enter_context(tc.tile_pool(name="const", bufs=1))
            hpool = ctx.enter_context(tc.tile_pool(name="hsb", bufs=2))
            epool = ctx.enter_context(tc.tile_pool(name="ep", bufs=2))
            pmm = ctx.enter_context(tc.tile_pool(name="pmm", bufs=2, space="PSUM"))
            pzq = ctx.enter_context(tc.tile_pool(name="pzq", bufs=2, space="PSUM"))
            ptr = ctx.enter_context(tc.tile_pool(name="ptr", bufs=1, space="PSUM"))

            # ------------- persistent SBUF tiles -------------
            dWs = [cpool.tile([128, QSTEPS * 48], f32, tag=f"dw{q}", name=f"dws{q}") for q in range(NQ)]
            dZs = [cpool.tile([128, QSTEPS * 48], f32, tag=f"dz{q}", name=f"dzs{q}") for q in range(NQ)]
            swp = cpool.tile([128, nsteps * 16], f32, tag="swp")
            L1b_bf = [cpool.tile([GCH, GCH * 128], bf16, tag=f"l1b{g}", name=f"l1bbf{g}") for g in range(NG)]
            W2bd_bf = cpool.tile([128, 128], bf16, tag="w2bd")
            W3_bf = cpool.tile([128, 4], bf16, tag="w3")
            W3_f = cpool.tile([128, 4], f32, tag="w3f")
            b1tab = cpool.tile([128, nsteps], f32, tag="b1tab")
            W1cT_sb = cpool.tile([128, 2], f32, tag="w1ct")
            trep = cpool.tile([128, nsteps], f32, tag="trep")
            b1c_sb = cpool.tile([128, 1], f32, tag="b1c")
            b2c_sb = cpool.tile([128, 1], f32, tag="b2c")
            b3s = cpool.tile([1, 4], f32, tag="b3s")
            b3f = cpool.tile([1, 4], f32, tag="b3f")
            b3rep = cpool.tile([1, 32], bf16, tag="b3rep")
            b3t32 = cpool.tile([128, 32], f32, tag="b3t32")
            ones_bf = cpool.tile([1, 128], bf16, tag="ones_bf")
            ones_col = cpool.tile([128, 1], f32, tag="ones_col")
            I128 = cpool.tile([128, 128], f32, tag="i128")
            I128_bf = cpool.tile([128, 128], bf16, tag="i128bf")
            y16 = [cpool.tile([GCH, 128], f32, tag=f"y16{g}", name=f"y16{g}") for g in range(NG)]
            ypad = [cpool.tile([GCH, 128], bf16, tag=f"ypad{g}", name=f"ypad{g}") for g in range(NG)]
            Y_f = cpool.tile([128, 16], f32, tag="Yf")
            loss_acc = cpool.tile([128, 16], f32, tag="loss_acc")
            ysq = [cpool.tile([GCH, 128], f32, tag=f"ysq{g}", name=f"ysq{g}") for g in range(NG)]
            ee = cpool.tile([128, 16], f32, tag="ee")
            loss_sb = cpool.tile([1, 16], f32, tag="loss_sb")
            loss1 = cpool.tile([1, 1], f32, tag="loss1")

            # ------------- init: DMAs -------------
            for q in range(NQ):
                nc.sync.dma_start(dWs[q][:], dWf_d[q][:])
                nc.sync.dma_start(dZs[q][:], dZf_d[q][:])
            for g in range(NG):
                nc.gpsimd.dma_start(L1b_bf[g][:], L1b_d[g][:])
            nc.gpsimd.dma_start(W2bd_bf[:], W2bd_d[:])
            nc.gpsimd.dma_start(ones_bf[:], ones_row_d[:])
            nc.gpsimd.dma_start(I128_bf[:], I128_d[:])
            nc.sync.dma_start(W3_f[:], W3c_d[:])
            nc.sync.dma_start(W1cT_sb[:], W1cT_d[:])
            nc.sync.dma_start(trep[:], trep_d[:])
            nc.sync.dma_start(b1c_sb[:], b1c_d[:])
            nc.sync.dma_start(b2c_sb[:], b2c_d[:])
            nc.sync.dma_start(b3f[:], b3c_d[:])
            nc.sync.dma_start(ones_col[:], ones_col_d[:])
            nc.sync.dma_start(I128[:], I128_d[:])
            for g in range(NG):
                nc.sync.dma_start(y16[g][:], y_init_d[g * GCH : (g + 1) * GCH, :])
            nc.sync.dma_start(Y_f[:], Y_init_d[:])

            # ------------- init: compute (no fp32 matmuls) -------------
            nc.vector.tensor_scalar(
                b1tab[:], trep[:], W1cT_sb[:, 0:1], b1c_sb[:, 0:1],
                op0=ALU.mult, op1=ALU.add,
            )
            nc.vector.tensor_scalar_mul(W3_bf[:, 0:3], W3_f[:, 0:3], float(SQRT_DT))
            nc.vector.tensor_scalar_mul(W3_bf[:, 3:4], W3_f[:, 3:4], float(DT))
            nc.vector.tensor_scalar_mul(b3s[0:1, 0:3], b3f[0:1, 0:3], float(SQRT_DT))
            nc.vector.tensor_scalar_mul(b3s[0:1, 3:4], b3f[0:1, 3:4], float(DT))
            nc.vector.tensor_copy(b3rep[0:1, 0:4], b3s[0:1, :])
            nc.vector.tensor_copy(b3rep[0:1, 4:8], b3rep[0:1, 0:4])
            nc.vector.tensor_copy(b3rep[0:1, 8:16], b3rep[0:1, 0:8])
            nc.vector.tensor_copy(b3rep[0:1, 16:32], b3rep[0:1, 0:16])
            # b3t32[p, :] = b3rep (partition broadcast via one bf16 matmul)
            b3ps = pzq.tile([128, 32], f32, tag="zq", name="b3ps")
            nc.tensor.matmul(b3ps[:], ones_bf[0:1, :], b3rep[0:1, :], start=True, stop=True)
            nc.scalar.copy(b3t32[:], b3ps[:])

            nc.vector.memset(loss_acc[:], 0.0)

            # sw prepass
            for q in range(NQ):
                nsq = max(0, min(nsteps, (q + 1) * QSTEPS) - q * QSTEPS)
                if nsq == 0:
                    continue
                lo = q * QSTEPS * 16
                src = dWs[q][:, 0 : nsq * 48].rearrange("p (s j) -> p s j", j=3)
                nc.vector.tensor_reduce(
                    swp[:, lo : lo + nsq * 16], src, axis=AX.X, op=ALU.add
                )
            nc.vector.tensor_scalar_mul(swp[:], swp[:], float(SIGMA0 * SQRT_DT))

            # ------------- time-step loop -------------
            SC_F = float((0.5 / DT) ** 0.5)
            for i in range(nsteps):
                qi, ri = divmod(i, QSTEPS)

                zq_sb = []
                h1ps = []
                h2ps = []
                h1sb = []
                h2sb = []
                # ---- per-group MLP chains (interleaved emission) ----
                for g in range(NG):
                    nc.vector.tensor_copy(ypad[g][:], y16[g][:])
                for g in range(NG):
                    t = pmm.tile([128, 1024], f32, tag="mm", name=f"h1ps{i}g{g}")
                    h1ps.append(t)
                    for c in range(GCH):
                        nc.tensor.matmul(
                            t[:, c * 128 : (c + 1) * 128],
                            L1b_bf[g][:, c * 128 : (c + 1) * 128],
                            ypad[g][:],
                            start=True,
                            stop=True,
                        )
                for g in range(NG):
                    h = hpool.tile([128, 1024], bf16, tag=f"h1{g}", name=f"h1sb{i}g{g}")
                    h1sb.append(h)
                    if g == 0:
                        nc.scalar.activation(
                            h[:], h1ps[g][:], AF.Relu, bias=b1tab[:, i : i + 1]
                        )
                    else:
                        nc.vector.tensor_scalar(
                            h[:], h1ps[g][:], b1tab[:, i : i + 1], 0.0,
                            op0=ALU.add, op1=ALU.max,
                        )
                for g in range(NG):
                    t = pmm.tile([128, 1024], f32, tag="mm", name=f"h2ps{i}g{g}")
                    h2ps.append(t)
                    for s in range(2):
                        nc.tensor.matmul(
                            t[:, s * 512 : (s + 1) * 512],
                            W2bd_bf[:],
                            h1sb[g][:, s * 512 : (s + 1) * 512],
                            start=True,
                            stop=True,
                        )
                for g in range(NG):
                    h = hpool.tile([128, 1024], bf16, tag=f"h2{g}", name=f"h2sb{i}g{g}")
                    h2sb.append(h)
                    if g == 0:
                        nc.vector.tensor_scalar(
                            h[:], h2ps[g][:], b2c_sb[:, 0:1], 0.0,
                            op0=ALU.add, op1=ALU.max,
                        )
                    else:
                        nc.scalar.activation(
                            h[:], h2ps[g][:], AF.Relu, bias=b2c_sb[:, 0:1]
                        )
                for g in range(NG):
                    zq_ps = pzq.tile([128, 32], f32, tag="zq", name=f"zqps{i}g{g}")
                    for c in range(GCH):
                        nc.tensor.matmul(
                            zq_ps[:, c * 4 : (c + 1) * 4],
                            h2sb[g][:, c * 128 : (c + 1) * 128],
                            W3_bf[:],
                            start=True,
                            stop=True,
                        )
                    z = epool.tile([128, 32], f32, tag=f"zqsb{g}", name=f"zqsb{i}g{g}")
                    zq_sb.append(z)
                    nc.vector.tensor_tensor(z[:], zq_ps[:], b3t32[:], op=ALU.add)

                # ---- per-group y recurrence ----
                tr_ps = ptr.tile([GCH, 256], bf16, tag="tr", name=f"tr{i}")
                incr = epool.tile([128, 16], bf16, tag="incr", name=f"incr{i}")
                for g in range(NG):
                    qview = zq_sb[g][:].rearrange("p (c m) -> p c m", m=4)[:, :, 3:4]
                    nc.gpsimd.tensor_tensor(
                        incr[:, g * 8 : (g + 1) * 8].rearrange("p (c o) -> p c o", o=1),
                        qview,
                        swp[:, i * 16 + g * 8 : i * 16 + (g + 1) * 8].rearrange(
                            "p (c o) -> p c o", o=1
                        ),
                        op=ALU.add,
                    )
                    nc.tensor.matmul(
                        tr_ps[:, g * 128 : (g + 1) * 128],
                        incr[:, g * 8 : (g + 1) * 8],
                        I128_bf[:],
                        is_transpose=True,
                    )
                    nc.vector.tensor_tensor(
                        y16[g][:], y16[g][:], tr_ps[:, g * 128 : (g + 1) * 128],
                        op=ALU.add,
                    )

                # ---- merged off-chain epilogue ----
                dwv = dWs[qi][:, ri * 48 : (ri + 1) * 48].rearrange("p (c j) -> p c j", j=3)
                dzv = dZs[qi][:, ri * 48 : (ri + 1) * 48].rearrange("p (c j) -> p c j", j=3)
                zz = epool.tile([128, 96], f32, tag="zz", name=f"zz{i}")
                uv = epool.tile([128, 32], f32, tag="uv", name=f"uv{i}")
                r_t = epool.tile([128, 16], f32, tag="r", name=f"r{i}")
                rr_t = epool.tile([128, 16], f32, tag="rr", name=f"rr{i}")
                fDT = epool.tile([128, 16], f32, tag="fdt", name=f"fdt{i}")
                umf = epool.tile([128, 16], f32, tag="umf", name=f"umf{i}")
                for g in range(NG):
                    zview = zq_sb[g][:].rearrange("p (c m) -> p c m", m=4)[:, :, 0:3]
                    qview = zq_sb[g][:].rearrange("p (c m) -> p c m", m=4)[:, :, 3:4]
                    nc.gpsimd.tensor_tensor(
                        zz[:, g * 24 : (g + 1) * 24].rearrange("p (c j) -> p c j", j=3),
                        zview,
                        dwv[:, g * 8 : (g + 1) * 8, :],
                        op=ALU.mult,
                    )
                    nc.gpsimd.tensor_tensor(
                        zz[:, 48 + g * 24 : 48 + (g + 1) * 24].rearrange(
                            "p (c j) -> p c j", j=3
                        ),
                        zview,
                        dzv[:, g * 8 : (g + 1) * 8, :],
                        op=ALU.mult,
                    )
                    nc.scalar.activation(
                        fDT[:, g * 8 : (g + 1) * 8].rearrange("p (c o) -> p c o", o=1),
                        qview,
                        AF.Square,
                        scale=SC_F,
                    )
                nc.vector.tensor_reduce(
                    uv[:],
                    zz[:].rearrange("p (h j) -> p h j", j=3),
                    axis=AX.X,
                    op=ALU.add,
                )
                nc.gpsimd.tensor_tensor(
                    r_t[:], uv[:, 0:16], uv[:, 16:32], op=ALU.subtract
                )
                nc.scalar.activation(rr_t[:], r_t[:], AF.Square)
                nc.gpsimd.tensor_tensor(loss_acc[:], loss_acc[:], rr_t[:], op=ALU.add)
                nc.gpsimd.tensor_tensor(umf[:], uv[:, 0:16], fDT[:], op=ALU.subtract)
                nc.gpsimd.tensor_tensor(Y_f[:], Y_f[:], umf[:], op=ALU.add)

            # ------------- terminal loss (fp32 matmuls OK here) -------------
            for g in range(NG):
                nc.scalar.activation(ysq[g][:], y16[g][:], AF.Square)
                ysq_ps = pzq.tile([128, GCH], f32, tag="zq", name=f"ysqps{g}")
                nc.tensor.matmul(
                    ysq_ps[:], ysq[g][:], I128[0:GCH, 0:GCH], is_transpose=True
                )
                nc.vector.tensor_tensor(
                    ee[:, g * 8 : (g + 1) * 8],
                    Y_f[:, g * 8 : (g + 1) * 8],
                    ysq_ps[:],
                    op=ALU.subtract,
                )
            nc.scalar.activation(ee[:], ee[:], AF.Square)
            nc.vector.tensor_tensor(loss_acc[:], loss_acc[:], ee[:], op=ALU.add)
            lps = ptr.tile([1, 16], f32, tag="lps", name="lps")
            nc.tensor.matmul(lps[:], ones_col[:], loss_acc[:], start=True, stop=True)
            nc.vector.tensor_copy(loss_sb[:], lps[:])
            nc.vector.tensor_reduce(
                loss1[:],
                loss_sb[0:1, :].rearrange("p (o c) -> p o c", o=1),
                axis=AX.X,
                op=ALU.add,
            )
            nc.vector.tensor_scalar_mul(loss1[:], loss1[:], 1.0 / B)
            nc.sync.dma_start(loss_out[:], loss1[:])
            if debug:
                for g in range(NG):
                    nc.sync.dma_start(y_out[g * GCH : (g + 1) * GCH, :], y16[g][:])
                nc.sync.dma_start(Y_out[:], Y_f[:])
                for g in range(NG):
                    nc.sync.dma_start(
                        zq_out[:, g * 32 : (g + 1) * 32], zq_sb[g][:]
                    )

    nc.compile()
    return nc


def _host_inputs(nsteps, y0, Y0, zW1, zb1, zW2, zb2, zW3, zb3, qW1, qb1, qW2, qb2, qW3, qb3, dW, dZ):
    """Per-core input maps. Layout/slicing only — no arithmetic on inputs."""
    f = np.float32
    QSTEPS = (nsteps + NQ - 1) // NQ
    W1row1 = np.concatenate([zW1[1], qW1[1]]).astype(f)  # (128,)
    L1bs = {}
    for g in range(NG):
        L1b = np.zeros((GCH, GCH * 128), f)
        for c in range(GCH):
            L1b[c, c * 128 : (c + 1) * 128] = W1row1
        L1bs[f"L1b{g}"] = L1b
    W1cT = np.ascontiguousarray(np.concatenate([zW1, qW1], axis=1).T).astype(f)
    W2bd = np.zeros((128, 128), f)
    W2bd[0:64, 0:64] = zW2
    W2bd[64:128, 64:128] = qW2
    W3c = np.zeros((128, 4), f)
    W3c[0:64, 0:3] = zW3
    W3c[64:128, 3] = qW3[:, 0]
    b1c = np.concatenate([zb1, qb1]).astype(f).reshape(128, 1)
    b2c = np.concatenate([zb2, qb2]).astype(f).reshape(128, 1)
    b3c = np.concatenate([zb3, qb3]).astype(f).reshape(1, 4)
    trep = np.broadcast_to((np.arange(nsteps) * DT).astype(f), (128, nsteps)).copy()
    ones_col = np.ones((128, 1), f)
    ones_row = np.ones((1, 128), f)
    I128 = np.eye(128, dtype=f)
    y_init = np.broadcast_to(np.asarray(y0, f).reshape(1, 1), (16, 128)).copy()
    Y_init = np.broadcast_to(np.asarray(Y0, f).reshape(1, 1), (128, 16)).copy()

    shared = dict(
        W1cT=W1cT, W2bd=W2bd, W3c=W3c, b1c=b1c, b2c=b2c, b3c=b3c,
        trep=trep, ones_col=ones_col, ones_row=ones_row, I128=I128,
        y_init=y_init, Y_init=Y_init, **L1bs,
    )

    in_maps = []
    for core in range(NCORES):
        o = core * BC
        m = dict(shared)
        for name, arr in (("dWf", dW), ("dZf", dZ)):
            x = np.ascontiguousarray(arr[:nsteps, o : o + BC, :]).astype(f)
            x = x.reshape(nsteps, NCH, 128, 3).transpose(2, 0, 1, 3)
            x = np.ascontiguousarray(x).reshape(128, nsteps * 48)
            for q in range(NQ):
                sl = x[:, q * QSTEPS * 48 : (q + 1) * QSTEPS * 48]
                buf = np.zeros((128, QSTEPS * 48), f)
                buf[:, : sl.shape[1]] = sl
                m[f"{name}{q}"] = buf
        in_maps.append(m)
    return in_maps


def _run(nsteps, inputs, debug=False):
    global LAST_EXEC_NS, LAST_RESULTS
    from concourse import bass_utils

    key = (nsteps, debug)
    if key not in _CACHE:
        _CACHE[key] = _build(nsteps, debug=debug)
    nc = _CACHE[key]

    in_maps = _host_inputs(nsteps, **inputs)
    trace = bool(os.environ.get("BASS_TRACE"))
    kwargs = {}
    if trace:
        import tempfile

        kwargs = dict(trace=True, tmpdir=tempfile.mkdtemp(prefix="bsde_trace_"))
    res = bass_utils.run_bass_kernel_spmd(
        nc, in_maps, core_ids=list(range(NCORES)), **kwargs
    )
    LAST_RESULTS = res
    LAST_EXEC_NS = res.exec_time_ns
    return res


def kernel(**inputs):
    inputs = {k: np.asarray(v, np.float32) for k, v in inputs.items()}
    res = _run(NSTEPS, inputs, debug=False)
    total = np.float32(0.0)
    for core in range(NCORES):
        total += res.results[core]["loss_out"][0, 0]
    return np.array(total, dtype=np.float32)
